# revision 1
# baseline (speedup 1.0000x reference)
"""NTM memory update (scatter_memory) on 8 Trainium2 NeuronCores.

Contract: kernel(**inputs) takes the FULL inputs (as produced by the problem's
setup), returns the FULL new_memory [4, 65536, 256] f32.

Strategy:
- The addressing pipeline (cosine over 16 representative rows -> scatter by
  std_ids -> interpolate -> 3-tap circular shift -> sharpen) only touches
  O(N) = 65536 floats per batch. It is computed on the host in float32,
  mirroring the reference ops exactly.
- The memory-regime part, new_memory = memory * (1 - w e^T) + w a^T, reads and
  writes 256 MiB each and runs on the 8 NeuronCores: shard = (batch, N-half),
  each core handles a [32768, 256] slice (32 MiB).
- Per 128-row block j with per-partition weight column w_j:
    P_j  = (e_bcast * (-w_j)) * mem_j          (DVE scalar_tensor_tensor)
    WA_j = a_bcast * w_j                       (ScalarE activation scale)
    out  = (P + WA) + mem                      (two DVE tensor_adds over
                                                [128, 2048] super-tiles)
"""

import numpy as np

B, N, M, C = 4, 65536, 256, 16
EPS = 1e-16
NCORES = 8
NSH = N // 2          # rows per core shard
P = 128               # SBUF partitions
JB = 8                # 128-row blocks per super-tile
NT = NSH // P         # 256 blocks per core
ST = NT // JB         # 32 super-tiles per core

_CACHE = {}


def _build():
    import concourse.bacc as bacc
    import concourse.tile as tile
    from concourse import mybir

    f32 = mybir.dt.float32
    Mul = mybir.AluOpType.mult

    nc = bacc.Bacc(
        "TRN2", target_bir_lowering=False, debug=False, num_devices=NCORES
    )
    mem = nc.dram_tensor("mem", [NSH, M], f32, kind="ExternalInput").ap()
    wt = nc.dram_tensor("wt", [P, NT], f32, kind="ExternalInput").ap()
    nwt = nc.dram_tensor("nwt", [P, NT], f32, kind="ExternalInput").ap()
    eb = nc.dram_tensor("eb", [P, M], f32, kind="ExternalInput").ap()
    ab = nc.dram_tensor("ab", [P, M], f32, kind="ExternalInput").ap()
    out = nc.dram_tensor("out", [NSH, M], f32, kind="ExternalOutput").ap()

    memv = mem.rearrange("(s j p) m -> s p j m", j=JB, p=P)
    outv = out.rearrange("(s j p) m -> s p j m", j=JB, p=P)

    with tile.TileContext(nc) as tc:
        with (
            tc.tile_pool(name="const", bufs=1) as const_pool,
            tc.tile_pool(name="memp", bufs=3) as mem_pool,
            tc.tile_pool(name="work", bufs=3) as work_pool,
            tc.tile_pool(name="outp", bufs=3) as out_pool,
        ):
            W = const_pool.tile([P, NT], f32)
            nc.sync.dma_start(W[:], wt[:])
            NW = const_pool.tile([P, NT], f32)
            nc.sync.dma_start(NW[:], nwt[:])
            EB = const_pool.tile([P, M], f32)
            nc.sync.dma_start(EB[:], eb[:])
            AB = const_pool.tile([P, M], f32)
            nc.sync.dma_start(AB[:], ab[:])

            for s in range(ST):
                mt = mem_pool.tile([P, JB, M], f32)
                nc.sync.dma_start(mt[:], memv[s])
                wa = work_pool.tile([P, JB, M], f32)
                pp = work_pool.tile([P, JB, M], f32)
                qq = work_pool.tile([P, JB, M], f32)
                ot = out_pool.tile([P, JB, M], f32)
                for j in range(JB):
                    t = s * JB + j
                    # WA_j = a * w_j   (ScalarE, per-partition scale)
                    nc.scalar.mul(wa[:, j, :], AB[:], W[:, t : t + 1])
                    # P_j = (e * -w_j) * mem_j   (DVE fused)
                    nc.vector.scalar_tensor_tensor(
                        pp[:, j, :],
                        EB[:],
                        NW[:, t : t + 1],
                        mt[:, j, :],
                        op0=Mul,
                        op1=Mul,
                    )
                nc.vector.tensor_add(qq[:], pp[:], wa[:])
                nc.vector.tensor_add(ot[:], qq[:], mt[:])
                nc.sync.dma_start(outv[s], ot[:])

    nc.compile()
    return nc


def _host_w(memory, k, beta, g, s, gamma, w_prev, std_ids, repre_ids):
    """Addressing pipeline in float32, mirroring the reference op-for-op."""
    f = np.float32
    memory = np.asarray(memory, f)
    k = np.asarray(k, f)
    beta = np.asarray(beta, f)
    g = np.asarray(g, f)
    s = np.asarray(s, f)
    gamma = np.asarray(gamma, f)
    w_prev = np.asarray(w_prev, f)
    std_ids = np.asarray(std_ids)
    repre_ids = np.asarray(repre_ids)

    eps = f(EPS)
    repre = memory[:, repre_ids, :]                          # [B, C, M]
    num = ((repre + eps) * (k[:, None, :] + eps)).sum(-1)    # [B, C]
    den = np.maximum(
        np.linalg.norm(repre + eps, axis=-1)
        * np.linalg.norm(k + eps, axis=-1, keepdims=True),
        f(1e-8),
    ).astype(f)
    cos = beta * np.maximum(num / den, f(0.0)) + eps         # [B, C]
    wc = cos[:, std_ids]                                     # [B, N]
    wg = g * wc + (f(1.0) - g) * w_prev
    wp = np.concatenate([wg[:, -1:], wg, wg[:, :1]], axis=1)
    w_sh = (
        s[:, 0:1] * wp[:, :N]
        + s[:, 1:2] * wp[:, 1 : N + 1]
        + s[:, 2:3] * wp[:, 2 : N + 2]
    )
    wpow = (w_sh.astype(f)) ** gamma
    w = wpow / (wpow.sum(axis=1, keepdims=True) + eps)
    return w.astype(f)                                       # [B, N]


def kernel(memory, k, beta, g, s, gamma, w_prev, e, a, std_ids, repre_ids):
    from concourse.bass_utils import run_bass_kernel_spmd

    memory = np.asarray(memory, np.float32)
    e = np.asarray(e, np.float32)
    a = np.asarray(a, np.float32)

    w = _host_w(memory, k, beta, g, s, gamma, w_prev, std_ids, repre_ids)

    if "nc" not in _CACHE:
        _CACHE["nc"] = _build()
    nc = _CACHE["nc"]

    in_maps = []
    for c in range(NCORES):
        b, h = divmod(c, 2)
        mem_shard = np.ascontiguousarray(memory[b, h * NSH : (h + 1) * NSH, :])
        w_shard = w[b, h * NSH : (h + 1) * NSH]
        wt = np.ascontiguousarray(w_shard.reshape(NT, P).T)
        in_maps.append(
            {
                "mem": mem_shard,
                "wt": wt,
                "nwt": np.ascontiguousarray(-wt),
                "eb": np.ascontiguousarray(
                    np.broadcast_to(e[b], (P, M))
                ),
                "ab": np.ascontiguousarray(
                    np.broadcast_to(a[b], (P, M))
                ),
            }
        )

    res = run_bass_kernel_spmd(nc, in_maps, core_ids=list(range(NCORES)))
    _CACHE["last_result"] = res

    new_memory = np.empty((B, N, M), np.float32)
    for c in range(NCORES):
        b, h = divmod(c, 2)
        new_memory[b, h * NSH : (h + 1) * NSH, :] = res.results[c]["out"]
    return new_memory


# revision 2
# speedup vs baseline: 1.0988x; 1.0988x over previous
"""NTM memory update (scatter_memory) on 8 Trainium2 NeuronCores.

Contract: kernel(**inputs) takes the FULL inputs (as produced by the problem's
setup), returns the FULL new_memory [4, 65536, 256] f32.

Strategy:
- The addressing pipeline (cosine over 16 representative rows -> scatter by
  std_ids -> interpolate -> 3-tap circular shift -> sharpen) only touches
  O(N) floats per batch; computed on host in float32 mirroring the reference.
- The memory-regime part, new_memory = memory * (1 - w e^T) + w a^T, moves
  512 MiB through HBM and runs on the 8 cores: shard = (batch, N-half),
  [32768, 256] f32 per core.
- Per super-tile of 1024 rows ([128 partitions, 8 blocks, 256]):
    PE (bf16):  C1 = 1 - w x e   -> PSUM (4 banks)   K=9 matmuls
                C2 =     w x a   -> PSUM (4 banks)   same stationary lhsT
    ACT:        C2 -> SBUF (f32 copy)
    DVE:        U  = mem * C1    (tensor_tensor, PSUM operand)
                out = U + C2     (tensor_tensor)
  The w x {e,a} terms are ~1e-4 * [0,1] corrections to memory, so bf16
  operands in the outer products contribute < 1e-6 relative output error.
"""

import numpy as np

B, N, M, C = 4, 65536, 256, 16
EPS = 1e-16
NCORES = 8
NSH = N // 2          # rows per core shard
P = 128               # SBUF partitions
JB = 8                # 128-row blocks per super-tile
NT = NSH // P         # 256 blocks per core
ST = NT // JB         # 32 super-tiles per core
K = 1 + JB            # matmul contraction: ones row + one w row per block
F = JB * M            # free size per super-tile (2048)

_CACHE = {}


def _build():
    import concourse.bacc as bacc
    import concourse.tile as tile
    from concourse import mybir

    f32 = mybir.dt.float32
    bf16 = mybir.dt.bfloat16

    nc = bacc.Bacc(
        "TRN2", target_bir_lowering=False, debug=False, num_devices=NCORES
    )
    mem = nc.dram_tensor("mem", [NSH, M], f32, kind="ExternalInput").ap()
    lhs = nc.dram_tensor("lhs", [ST, K, P], bf16, kind="ExternalInput").ap()
    rhs = nc.dram_tensor("rhs", [K, 2 * F], bf16, kind="ExternalInput").ap()
    out = nc.dram_tensor("out", [NSH, M], f32, kind="ExternalOutput").ap()

    memv = mem.rearrange("(s j p) m -> s p j m", j=JB, p=P)
    outv = out.rearrange("(s j p) m -> s p j m", j=JB, p=P)

    NMM = F // 512  # 512-wide matmuls per C tile

    with tile.TileContext(nc) as tc:
        with (
            tc.tile_pool(name="const", bufs=1) as cpool,
            tc.tile_pool(name="lhsp", bufs=4) as lpool,
            tc.tile_pool(name="memp", bufs=3) as mpool,
            tc.tile_pool(name="c2p", bufs=3) as c2pool,
            tc.tile_pool(name="outp", bufs=3) as opool,
            tc.tile_pool(name="ps1", bufs=1, space="PSUM") as ps1pool,
            tc.tile_pool(name="ps2", bufs=1, space="PSUM") as ps2pool,
        ):
            R = cpool.tile([K, 2 * F], bf16)
            nc.sync.dma_start(R[:], rhs[:])

            for s in range(ST):
                mt = mpool.tile([P, JB, M], f32)
                nc.sync.dma_start(mt[:], memv[s])
                L = lpool.tile([K, P], bf16)
                nc.sync.dma_start(L[:], lhs[s])

                c1 = ps1pool.tile([P, F], f32)
                c2 = ps2pool.tile([P, F], f32)
                for q in range(NMM):
                    sl = slice(q * 512, (q + 1) * 512)
                    nc.tensor.matmul(c1[:, sl], L[:], R[:, sl])
                for q in range(NMM):
                    sl = slice(q * 512, (q + 1) * 512)
                    nc.tensor.matmul(
                        c2[:, sl], L[:], R[:, F + q * 512 : F + (q + 1) * 512]
                    )

                c2s = c2pool.tile([P, F], f32)
                nc.scalar.copy(c2s[:], c2[:])

                ot = opool.tile([P, JB, M], f32)
                otf = ot.rearrange("p j m -> p (j m)")
                mtf = mt.rearrange("p j m -> p (j m)")
                nc.vector.tensor_mul(otf, mtf, c1[:])
                nc.vector.tensor_add(otf, otf, c2s[:])
                nc.scalar.dma_start(outv[s], ot[:])

    nc.compile()
    return nc


def _host_w(memory, k, beta, g, s, gamma, w_prev, std_ids, repre_ids):
    """Addressing pipeline in float32, mirroring the reference op-for-op."""
    f = np.float32
    memory = np.asarray(memory, f)
    k = np.asarray(k, f)
    beta = np.asarray(beta, f)
    g = np.asarray(g, f)
    s = np.asarray(s, f)
    gamma = np.asarray(gamma, f)
    w_prev = np.asarray(w_prev, f)
    std_ids = np.asarray(std_ids)
    repre_ids = np.asarray(repre_ids)

    eps = f(EPS)
    repre = memory[:, repre_ids, :]                          # [B, C, M]
    num = ((repre + eps) * (k[:, None, :] + eps)).sum(-1)    # [B, C]
    den = np.maximum(
        np.linalg.norm(repre + eps, axis=-1)
        * np.linalg.norm(k + eps, axis=-1, keepdims=True),
        f(1e-8),
    ).astype(f)
    cos = beta * np.maximum(num / den, f(0.0)) + eps         # [B, C]
    wc = cos[:, std_ids]                                     # [B, N]
    wg = g * wc + (f(1.0) - g) * w_prev
    wp = np.concatenate([wg[:, -1:], wg, wg[:, :1]], axis=1)
    w_sh = (
        s[:, 0:1] * wp[:, :N]
        + s[:, 1:2] * wp[:, 1 : N + 1]
        + s[:, 2:3] * wp[:, 2 : N + 2]
    )
    wpow = (w_sh.astype(f)) ** gamma
    w = wpow / (wpow.sum(axis=1, keepdims=True) + eps)
    return w.astype(f)                                       # [B, N]


def kernel(memory, k, beta, g, s, gamma, w_prev, e, a, std_ids, repre_ids):
    import ml_dtypes
    from concourse.bass_utils import run_bass_kernel_spmd

    bf16 = ml_dtypes.bfloat16
    memory = np.asarray(memory, np.float32)
    e = np.asarray(e, np.float32)
    a = np.asarray(a, np.float32)

    w = _host_w(memory, k, beta, g, s, gamma, w_prev, std_ids, repre_ids)

    if "nc" not in _CACHE:
        _CACHE["nc"] = _build()
    nc = _CACHE["nc"]

    in_maps = []
    for c in range(NCORES):
        b, h = divmod(c, 2)
        mem_shard = np.ascontiguousarray(memory[b, h * NSH : (h + 1) * NSH, :])
        w_shard = w[b, h * NSH : (h + 1) * NSH]

        lhs = np.zeros((ST, K, P), np.float32)
        lhs[:, 0, :] = 1.0
        lhs[:, 1:, :] = w_shard.reshape(ST, JB, P)  # [s, j, p]

        rhs = np.zeros((K, 2 * F), np.float32)
        rhs[0, :F] = 1.0
        for j in range(JB):
            rhs[1 + j, j * M : (j + 1) * M] = -e[b]
            rhs[1 + j, F + j * M : F + (j + 1) * M] = a[b]

        in_maps.append(
            {
                "mem": mem_shard,
                "lhs": lhs.astype(bf16),
                "rhs": rhs.astype(bf16),
            }
        )

    res = run_bass_kernel_spmd(nc, in_maps, core_ids=list(range(NCORES)))
    _CACHE["last_result"] = res

    new_memory = np.empty((B, N, M), np.float32)
    for c in range(NCORES):
        b, h = divmod(c, 2)
        new_memory[b, h * NSH : (h + 1) * NSH, :] = res.results[c]["out"]
    return new_memory
